# revision 12
# baseline (speedup 1.0000x reference)
"""Trainium2 Bass kernel for DiagnosticAttention (B=2, L=2048, H=1024, NH=16).

Sharding: 8 cores = 2 batches (data-parallel) x 4 head-blocks (tensor-parallel,
4 heads each).  Each core computes, for its batch b and its 4 heads:
  Q^T,K^T (head-transposed), V (+gate cols), per-head attention in S^T layout
  (keys on partitions -> all score-bias terms are per-partition, softmax sum
  via an appended ones-column in V), and a partial out-projection with its
  256 rows of Wo.  The host sums the 4 partial outputs per batch and adds bo.
"""

import sys

for _p in ("/opt/trn_rl_repo", "/root/.axon_site/_ro/trn_rl_repo"):
    if _p not in sys.path:
        sys.path.insert(0, _p)

import numpy as np

B, L, H, NH = 2, 2048, 1024, 16
HD = H // NH            # 64
NCORES = 8
HPC = 4                 # heads per core
DPC = HPC * HD          # 256 head-dims per core
KT = H // 128           # 8 contraction tiles for projections
LT = L // 128           # 16 l tiles
CH = 512                # lq chunk
NCH = L // CH           # 4 chunks
HW_ = 65                    # per-head V block: 64 V cols + ones col
GW = HPC * HW_              # 260: start of gate cols
VW = GW + HPC               # 264 total

_RUNNER = None


def _build():
    import concourse.bass as bass
    import concourse.bacc as bacc
    import concourse.tile as tile
    from concourse import mybir

    F32 = mybir.dt.float32
    F32R = mybir.dt.float32r
    AL = mybir.AluOpType
    AF = mybir.ActivationFunctionType

    nc = bacc.Bacc(None, target_bir_lowering=False)

    xT = nc.dram_tensor("xT", [H, L], F32R, kind="ExternalInput")
    wq = nc.dram_tensor("wq", [H, DPC], F32R, kind="ExternalInput")
    wk = nc.dram_tensor("wk", [H, DPC], F32R, kind="ExternalInput")
    wvg = nc.dram_tensor("wvg", [H, VW], F32R, kind="ExternalInput")
    wo = nc.dram_tensor("wo", [DPC, H], F32R, kind="ExternalInput")
    bq = nc.dram_tensor("bq", [DPC], F32, kind="ExternalInput")
    bk = nc.dram_tensor("bk", [DPC], F32, kind="ExternalInput")
    bvg = nc.dram_tensor("bvg", [VW], F32, kind="ExternalInput")
    emask = nc.dram_tensor("emask", [128, LT], F32, kind="ExternalInput")
    md = nc.dram_tensor("md", [128, LT * HPC], F32, kind="ExternalInput")
    out = nc.dram_tensor("out", [L, H], F32, kind="ExternalOutput")
    csd = nc.dram_tensor("csd", [4, L], F32)
    rscr = nc.dram_tensor("rscr", [4, L], F32)

    with tile.TileContext(nc) as tc:
        with (
            tc.tile_pool(name="persist", bufs=1) as P1,
            tc.tile_pool(name="es", bufs=4) as ES,
            tc.tile_pool(name="rb", bufs=2) as RB,
            tc.tile_pool(name="psmm", bufs=2, space="PSUM") as PSMM,
            tc.tile_pool(name="pss", bufs=2, space="PSUM") as PSS,
            tc.tile_pool(name="pspv", bufs=2, space="PSUM") as PSPV,
        ):
            # ---- persistent SBUF tensors -------------------------------
            xt = [P1.tile([128, L], F32R, name=f"xt{k}") for k in range(KT)]
            wq_s = [P1.tile([128, DPC], F32R, name=f"wq{k}") for k in range(KT)]
            wk_s = [P1.tile([128, DPC], F32R, name=f"wk{k}") for k in range(KT)]
            wvg_s = [P1.tile([128, VW], F32R, name=f"wvg{k}") for k in range(KT)]
            wo_s = [P1.tile([64, H], F32R, name=f"wo{k}") for k in range(HPC)]
            qt = [P1.tile([128, L], F32R, name=f"qt{m}") for m in range(2)]
            kt = [P1.tile([128, L], F32R, name=f"kt{m}") for m in range(2)]
            v = [P1.tile([128, VW], F32R, name=f"v{t}") for t in range(LT)]
            bq_s = P1.tile([128, 2], F32, name="bqs")
            bk_s = P1.tile([128, 2], F32, name="bks")
            bvg_s = P1.tile([128, VW], F32, name="bvgs")
            em_s = P1.tile([128, LT], F32, name="ems")
            md_s = P1.tile([128, LT * HPC], F32, name="mds")
            bias_c = P1.tile([128, LT * HPC], F32, name="biasc")
            gp = P1.tile([128, HPC], F32, name="gp")
            gs = P1.tile([128, HPC], F32, name="gs")
            cs65 = P1.tile([65, L], F32, name="cs65")
            cst = P1.tile([128, 4 * LT], F32, name="cst")
            rt = P1.tile([128, 4 * LT], F32, name="rt")

            # ---- input DMAs --------------------------------------------
            for k in range(KT):
                nc.sync.dma_start(out=xt[k][:], in_=xT[128 * k:128 * (k + 1), :])
                nc.sync.dma_start(out=wq_s[k][:], in_=wq[128 * k:128 * (k + 1), :])
                nc.sync.dma_start(out=wk_s[k][:], in_=wk[128 * k:128 * (k + 1), :])
                nc.sync.dma_start(out=wvg_s[k][:], in_=wvg[128 * k:128 * (k + 1), :])
            for k in range(HPC):
                nc.sync.dma_start(out=wo_s[k][:], in_=wo[64 * k:64 * (k + 1), :])
            for k in range(2):
                nc.sync.dma_start(
                    out=bq_s[:, k:k + 1], in_=bq[128 * k:128 * (k + 1)][:, None])
                nc.sync.dma_start(
                    out=bk_s[:, k:k + 1], in_=bk[128 * k:128 * (k + 1)][:, None])
            nc.sync.dma_start(out=bvg_s[:], in_=bvg[None, :].to_broadcast((128, VW)))
            nc.sync.dma_start(out=em_s[:], in_=emask[:, :])
            nc.sync.dma_start(out=md_s[:], in_=md[:, :])

            # ---- Q^T / K^T projections ---------------------------------
            # qt[m][:, c] = (Wq[:, 128m:128m+128]).T @ xT chunk  + bq
            for wsrc, bsrc, dst in ((wq_s, bq_s, qt), (wk_s, bk_s, kt)):
                for m in range(2):
                    for c in range(NCH):
                        ps = PSMM.tile([128, CH], F32, name="mm")
                        for k in range(KT):
                            nc.tensor.matmul(
                                ps[:],
                                wsrc[k][:, 128 * m:128 * (m + 1)],
                                xt[k][:, CH * c:CH * (c + 1)],
                                start=(k == 0), stop=(k == KT - 1))
                        nc.vector.tensor_scalar_add(
                            dst[m][:, CH * c:CH * (c + 1)], ps[:],
                            bsrc[:, m:m + 1])

            # ---- V (+gate) projection, bias cols -----------------------
            for t in range(LT):
                ps = PSMM.tile([128, CH], F32, name="mm")
                pv = ps[:, 0:VW]
                for k in range(KT):
                    nc.tensor.matmul(
                        pv,
                        xt[k][:, 128 * t:128 * (t + 1)],
                        wvg_s[k][:],
                        start=(k == 0), stop=(k == KT - 1))
                # gate: sigmoid(x@Wg + bg), then bias col = em*gate + md
                nc.vector.tensor_add(
                    gp[:], ps[:, GW:VW], bvg_s[:, GW:VW])
                nc.scalar.activation(gs[:], gp[:], AF.Sigmoid)
                nc.vector.scalar_tensor_tensor(
                    out=bias_c[:, HPC * t:HPC * (t + 1)],
                    in0=gs[:], scalar=em_s[:, t:t + 1],
                    in1=md_s[:, HPC * t:HPC * (t + 1)],
                    op0=AL.mult, op1=AL.add)
                # V(+bias) -> SBUF, then ones column per head
                nc.vector.tensor_add(
                    v[t][:, 0:GW], pv[:, 0:GW], bvg_s[:, 0:GW])
                # ones columns come via bvg (host sets bvg[65h+64]=1)

            # ---- attention, S^T layout ---------------------------------
            # per-head (PV)^T accumulators reuse the (now dead) xt slots
            ot = [P1.tile([64, L], F32R, name=f"xt{h}") for h in range(HPC)]
            for h in range(HPC):
                mt, hf = h // 2, 64 * (h % 2)
                for c in range(NCH):
                    pv = PSPV.tile([128, CH], F32, name="pv")
                    for m in range(LT):
                        ss = PSS.tile([128, CH], F32, name="ss")
                        nc.tensor.matmul(
                            ss[:],
                            kt[mt][hf:hf + 64, 128 * m:128 * (m + 1)],
                            qt[mt][hf:hf + 64, CH * c:CH * (c + 1)],
                            start=True, stop=True)
                        es = ES.tile([128, CH], F32R, name="es")
                        nc.scalar.activation(
                            es[:], ss[:], AF.Exp,
                            bias=bias_c[:, HPC * m + h:HPC * m + h + 1],
                            scale=1.0 / np.sqrt(HD).item())
                        nc.tensor.matmul(
                            pv[0:HD + 1, :],
                            v[m][:, HW_ * h:HW_ * (h + 1)],
                            es[:],
                            start=(m == 0), stop=(m == LT - 1))
                    # colsum row (partition 64); (PV)^T rows -> ot
                    nc.vector.tensor_copy(
                        cs65[HD:HD + 1, CH * c:CH * (c + 1)], pv[HD:HD + 1, :])
                    nc.vector.tensor_copy(
                        ot[h][0:HD, CH * c:CH * (c + 1)], pv[0:HD, :])
                # this head's colsum row -> DRAM before next head reuses cs65
                nc.sync.dma_start(out=csd[h, :][None, :], in_=cs65[HD:HD + 1, :])

            # ---- softmax normalizers -----------------------------------
            # cst[p, 16h+i] = csd[h, 128i+p]   (transposing DMA per head)
            for h in range(HPC):
                dsrc = csd[h, :][None, :].rearrange("a (i q) -> a q i", q=128)
                nc.sync.dma_start(out=cst[:, 16 * h:16 * (h + 1)], in_=dsrc)
            nc.vector.reciprocal(rt[:], cst[:])
            # rscr[h, 128i+p] = rt[p, 16h+i]   (transposing DMA per head)
            for h in range(HPC):
                ddst = rscr[h, :][None, :].rearrange("a (i q) -> a q i", q=128)
                nc.sync.dma_start(out=ddst, in_=rt[:, 16 * h:16 * (h + 1)])
            for h in range(HPC):
                rb = RB.tile([64, L], F32, name="rb")
                nc.sync.dma_start(out=rb[:], in_=rscr[h, :][None, :].to_broadcast((64, L)))
                for c in range(NCH):
                    sl = ot[h][0:HD, CH * c:CH * (c + 1)]
                    nc.vector.tensor_mul(sl, sl, rb[:, CH * c:CH * (c + 1)])

            # ---- out-projection (partial over this core's 256 dims) ----
            stage_tags = ["qt0", "qt1", "kt0", "kt1"]
            for t in range(LT):
                for n in range(2):
                    ps = PSMM.tile([128, CH], F32, name="mm")
                    for k in range(HPC):
                        nc.tensor.matmul(
                            ps[:],
                            ot[k][0:HD, 128 * t:128 * (t + 1)],
                            wo_s[k][0:HD, CH * n:CH * (n + 1)],
                            start=(k == 0), stop=(k == HPC - 1))
                    idx = 2 * t + n               # 0..31
                    stage = P1.tile([128, CH], F32, name=stage_tags[idx % 4])
                    nc.scalar.copy(stage[:], ps[:])
                    nc.sync.dma_start(
                        out=out[128 * t:128 * (t + 1), CH * n:CH * (n + 1)],
                        in_=stage[:])

    nc.finalize()
    return nc


def _make_runner():
    """Compile once; return f(in_maps) -> list of per-core output dicts.

    Same execution path as concourse.bass_utils.run_bass_kernel_spmd under
    axon (bass2jax custom-call via PJRT), but with the jitted executable
    cached so repeated calls don't recompile.
    """
    import jax
    from jax.experimental.shard_map import shard_map
    from jax.sharding import Mesh, PartitionSpec
    from concourse import bass2jax, mybir

    nc = _build()
    bass2jax.install_neuronx_cc_hook()

    partition_name = nc.partition_id_tensor.name if nc.partition_id_tensor else None
    in_names, out_names, out_avals, zero_outs = [], [], [], []
    for alloc in nc.m.functions[0].allocations:
        if not isinstance(alloc, mybir.MemoryLocationSet):
            continue
        name = alloc.memorylocations[0].name
        if alloc.kind == "ExternalInput":
            if name != partition_name:
                in_names.append(name)
        elif alloc.kind == "ExternalOutput":
            out_names.append(name)
            shape = tuple(alloc.tensor_shape)
            dtype = mybir.dt.np(alloc.dtype)
            out_avals.append(jax.core.ShapedArray(shape, dtype))
            zero_outs.append(np.zeros(shape, dtype))
    n_params = len(in_names)
    n_outs = len(out_avals)
    feed_names = list(in_names) + list(out_names)
    if partition_name is not None:
        feed_names.append(partition_name)
    donate = tuple(range(n_params, n_params + n_outs))

    def _body(*args):
        operands = list(args)
        if partition_name is not None:
            operands.append(bass2jax.partition_id_tensor())
        outs = bass2jax._bass_exec_p.bind(
            *operands,
            out_avals=tuple(out_avals),
            in_names=tuple(feed_names),
            out_names=tuple(out_names),
            lowering_input_output_aliases=(),
            sim_require_finite=True,
            sim_require_nnan=True,
            nc=nc,
        )
        return tuple(outs)

    devices = jax.devices()[:NCORES]
    mesh = Mesh(np.asarray(devices), ("core",))
    sharded = jax.jit(
        shard_map(
            _body, mesh=mesh,
            in_specs=(PartitionSpec("core"),) * (n_params + n_outs),
            out_specs=(PartitionSpec("core"),) * n_outs,
            check_rep=False,
        ),
        donate_argnums=donate, keep_unused=True,
    )

    def run(in_maps):
        gi = [np.concatenate([np.asarray(m[nm]) for m in in_maps], axis=0)
              for nm in in_names]
        go = [np.concatenate([z] * NCORES, axis=0) for z in zero_outs]
        outs = sharded(*gi, *go)
        res = []
        for i in range(NCORES):
            d = {}
            for j, nm in enumerate(out_names):
                n0 = zero_outs[j].shape[0]
                d[nm] = np.asarray(outs[j][i * n0:(i + 1) * n0])
            res.append(d)
        return res

    return run


def _shard_inputs(hidden_states, attention_mask, has_error_codes,
                  Wq, bq, Wk, bk, Wv, bv, Wo, bo, diag_bias, Wg, bg):
    f32 = np.float32
    hs = np.asarray(hidden_states, f32)
    am = np.asarray(attention_mask, f32).reshape(B, L)
    ec = np.asarray(has_error_codes).astype(f32)
    Wq, Wk, Wv, Wo = (np.asarray(w, f32) for w in (Wq, Wk, Wv, Wo))
    Wg = np.asarray(Wg, f32)
    bq, bk, bv, bg = (np.asarray(x, f32) for x in (bq, bk, bv, bg))
    diag = np.asarray(diag_bias, f32).reshape(NH)

    in_maps = []
    for core in range(NCORES):
        b, hb = core // 4, core % 4
        heads = range(4 * hb, 4 * hb + 4)
        cols = slice(DPC * hb, DPC * (hb + 1))
        wvg = np.zeros((H, VW), f32)
        bvg = np.zeros((VW,), f32)
        for j, h in enumerate(heads):
            wvg[:, HW_ * j:HW_ * j + HD] = Wv[:, HD * h:HD * (h + 1)]
            bvg[HW_ * j:HW_ * j + HD] = bv[HD * h:HD * (h + 1)]
            wvg[:, GW + j] = Wg[:, h]
            bvg[GW + j] = bg[h]
            bvg[HW_ * j + HD] = 1.0
        mdv = am[b][:, None] + diag[list(heads)][None, :]          # (L, 4)
        in_maps.append({
            "xT": np.ascontiguousarray(hs[b].T),
            "wq": np.ascontiguousarray(Wq[:, cols]),
            "wk": np.ascontiguousarray(Wk[:, cols]),
            "wvg": wvg,
            "wo": np.ascontiguousarray(Wo[cols, :]),
            "bq": np.ascontiguousarray(bq[cols]),
            "bk": np.ascontiguousarray(bk[cols]),
            "bvg": bvg,
            "emask": np.ascontiguousarray(ec[b].reshape(LT, 128).T),
            "md": np.ascontiguousarray(
                mdv.reshape(LT, 128, HPC).transpose(1, 0, 2).reshape(128, LT * HPC)),
        })
    return in_maps


def kernel(**inputs) -> np.ndarray:
    global _RUNNER
    if _RUNNER is None:
        _RUNNER = _make_runner()
    in_maps = _shard_inputs(**inputs)
    results = _RUNNER(in_maps)
    bo = np.asarray(inputs["bo"], np.float32)
    out = np.zeros((B, L, H), np.float32)
    for b in range(B):
        acc = np.zeros((L, H), np.float64)
        for j in range(4):
            acc += results[4 * b + j]["out"].astype(np.float64)
        out[b] = (acc + bo.astype(np.float64)).astype(np.float32)
    return out


# revision 13
# speedup vs baseline: 6996.2825x; 6996.2825x over previous
"""Trainium2 Bass kernel for DiagnosticAttention (B=2, L=2048, H=1024, NH=16).

Sharding: 8 cores = 2 batches (data-parallel) x 4 head-blocks (tensor-parallel,
4 heads each).  Each core computes, for its batch b and its 4 heads:
  Q^T,K^T (head-transposed), V (+gate cols), per-head attention in S^T layout
  (keys on partitions -> all score-bias terms are per-partition, softmax sum
  via an appended ones-column in V), and a partial out-projection with its
  256 rows of Wo.  The host sums the 4 partial outputs per batch and adds bo.
"""

import sys

for _p in ("/opt/trn_rl_repo", "/root/.axon_site/_ro/trn_rl_repo"):
    if _p not in sys.path:
        sys.path.insert(0, _p)

import numpy as np

B, L, H, NH = 2, 2048, 1024, 16
HD = H // NH            # 64
NCORES = 8
HPC = 4                 # heads per core
DPC = HPC * HD          # 256 head-dims per core
KT = H // 128           # 8 contraction tiles for projections
LT = L // 128           # 16 l tiles
CH = 512                # lq chunk
NCH = L // CH           # 4 chunks
HW_ = 65                    # per-head V block: 64 V cols + ones col
GW = HPC * HW_              # 260: start of gate cols
VW = GW + HPC               # 264 total

_RUNNER = None


def _build():
    import concourse.bass as bass
    import concourse.bacc as bacc
    import concourse.tile as tile
    from concourse import mybir

    F32 = mybir.dt.float32
    F32R = mybir.dt.float32r
    AL = mybir.AluOpType
    AF = mybir.ActivationFunctionType

    nc = bacc.Bacc(None, target_bir_lowering=False)

    xT = nc.dram_tensor("xT", [H, L], F32R, kind="ExternalInput")
    wq = nc.dram_tensor("wq", [H, DPC], F32R, kind="ExternalInput")
    wk = nc.dram_tensor("wk", [H, DPC], F32R, kind="ExternalInput")
    wvg = nc.dram_tensor("wvg", [H, VW], F32R, kind="ExternalInput")
    wo = nc.dram_tensor("wo", [DPC, H], F32R, kind="ExternalInput")
    bq = nc.dram_tensor("bq", [DPC], F32, kind="ExternalInput")
    bk = nc.dram_tensor("bk", [DPC], F32, kind="ExternalInput")
    bvg = nc.dram_tensor("bvg", [VW], F32, kind="ExternalInput")
    emask = nc.dram_tensor("emask", [128, LT], F32, kind="ExternalInput")
    md = nc.dram_tensor("md", [128, LT * HPC], F32, kind="ExternalInput")
    out = nc.dram_tensor("out", [L, H], F32, kind="ExternalOutput")
    csd = nc.dram_tensor("csd", [4, L], F32)
    rscr = nc.dram_tensor("rscr", [4, L], F32)

    with tile.TileContext(nc) as tc:
        with (
            tc.tile_pool(name="persist", bufs=1) as P1,
            tc.tile_pool(name="es", bufs=4) as ES,
            tc.tile_pool(name="rb", bufs=2) as RB,
            tc.tile_pool(name="psmm", bufs=2, space="PSUM") as PSMM,
            tc.tile_pool(name="pss", bufs=2, space="PSUM") as PSS,
            tc.tile_pool(name="pspv", bufs=2, space="PSUM") as PSPV,
        ):
            # ---- persistent SBUF tensors -------------------------------
            xt = [P1.tile([128, L], F32R, name=f"xt{k}") for k in range(KT)]
            wq_s = [P1.tile([128, DPC], F32R, name=f"wq{k}") for k in range(KT)]
            wk_s = [P1.tile([128, DPC], F32R, name=f"wk{k}") for k in range(KT)]
            wvg_s = [P1.tile([128, VW], F32R, name=f"wvg{k}") for k in range(KT)]
            wo_s = [P1.tile([64, H], F32R, name=f"wo{k}") for k in range(HPC)]
            qt = [P1.tile([128, L], F32R, name=f"qt{m}") for m in range(2)]
            kt = [P1.tile([128, L], F32R, name=f"kt{m}") for m in range(2)]
            v = [P1.tile([128, VW], F32R, name=f"v{t}") for t in range(LT)]
            bq_s = P1.tile([128, 2], F32, name="bqs")
            bk_s = P1.tile([128, 2], F32, name="bks")
            bvg_s = P1.tile([128, VW], F32, name="bvgs")
            em_s = P1.tile([128, LT], F32, name="ems")
            md_s = P1.tile([128, LT * HPC], F32, name="mds")
            bias_c = P1.tile([128, LT * HPC], F32, name="biasc")
            gp = P1.tile([128, HPC], F32, name="gp")
            gs = P1.tile([128, HPC], F32, name="gs")
            cs65 = P1.tile([65, L], F32, name="cs65")
            cst = P1.tile([128, 4 * LT], F32, name="cst")
            rt = P1.tile([128, 4 * LT], F32, name="rt")

            # ---- input DMAs --------------------------------------------
            for k in range(KT):
                nc.sync.dma_start(out=xt[k][:], in_=xT[128 * k:128 * (k + 1), :])
                nc.sync.dma_start(out=wq_s[k][:], in_=wq[128 * k:128 * (k + 1), :])
                nc.sync.dma_start(out=wk_s[k][:], in_=wk[128 * k:128 * (k + 1), :])
                nc.sync.dma_start(out=wvg_s[k][:], in_=wvg[128 * k:128 * (k + 1), :])
            for k in range(HPC):
                nc.sync.dma_start(out=wo_s[k][:], in_=wo[64 * k:64 * (k + 1), :])
            for k in range(2):
                nc.sync.dma_start(
                    out=bq_s[:, k:k + 1], in_=bq[128 * k:128 * (k + 1)][:, None])
                nc.sync.dma_start(
                    out=bk_s[:, k:k + 1], in_=bk[128 * k:128 * (k + 1)][:, None])
            nc.sync.dma_start(out=bvg_s[:], in_=bvg[None, :].to_broadcast((128, VW)))
            nc.sync.dma_start(out=em_s[:], in_=emask[:, :])
            nc.sync.dma_start(out=md_s[:], in_=md[:, :])

            # ---- Q^T / K^T projections ---------------------------------
            # qt[m][:, c] = (Wq[:, 128m:128m+128]).T @ xT chunk  + bq
            for wsrc, bsrc, dst in ((wq_s, bq_s, qt), (wk_s, bk_s, kt)):
                for m in range(2):
                    for c in range(NCH):
                        ps = PSMM.tile([128, CH], F32, name="mm")
                        for k in range(KT):
                            nc.tensor.matmul(
                                ps[:],
                                wsrc[k][:, 128 * m:128 * (m + 1)],
                                xt[k][:, CH * c:CH * (c + 1)],
                                start=(k == 0), stop=(k == KT - 1))
                        nc.vector.tensor_scalar_add(
                            dst[m][:, CH * c:CH * (c + 1)], ps[:],
                            bsrc[:, m:m + 1])

            # ---- V (+gate) projection, bias cols -----------------------
            for t in range(LT):
                ps = PSMM.tile([128, CH], F32, name="mm")
                pv = ps[:, 0:VW]
                for k in range(KT):
                    nc.tensor.matmul(
                        pv,
                        xt[k][:, 128 * t:128 * (t + 1)],
                        wvg_s[k][:],
                        start=(k == 0), stop=(k == KT - 1))
                # gate: sigmoid(x@Wg + bg), then bias col = em*gate + md
                nc.vector.tensor_add(
                    gp[:], ps[:, GW:VW], bvg_s[:, GW:VW])
                nc.scalar.activation(gs[:], gp[:], AF.Sigmoid)
                nc.vector.scalar_tensor_tensor(
                    out=bias_c[:, HPC * t:HPC * (t + 1)],
                    in0=gs[:], scalar=em_s[:, t:t + 1],
                    in1=md_s[:, HPC * t:HPC * (t + 1)],
                    op0=AL.mult, op1=AL.add)
                # V(+bias) -> SBUF, then ones column per head
                nc.vector.tensor_add(
                    v[t][:, 0:GW], pv[:, 0:GW], bvg_s[:, 0:GW])
                # ones columns come via bvg (host sets bvg[65h+64]=1)

            # ---- attention, S^T layout ---------------------------------
            # per-head (PV)^T accumulators reuse the (now dead) xt slots
            ot = [P1.tile([64, L], F32R, name=f"xt{h}") for h in range(HPC)]
            for h in range(HPC):
                mt, hf = h // 2, 64 * (h % 2)
                for c in range(NCH):
                    pv = PSPV.tile([128, CH], F32, name="pv")
                    for m in range(LT):
                        ss = PSS.tile([128, CH], F32, name="ss")
                        nc.tensor.matmul(
                            ss[:],
                            kt[mt][hf:hf + 64, 128 * m:128 * (m + 1)],
                            qt[mt][hf:hf + 64, CH * c:CH * (c + 1)],
                            start=True, stop=True)
                        es = ES.tile([128, CH], F32R, name="es")
                        nc.scalar.activation(
                            es[:], ss[:], AF.Exp,
                            bias=bias_c[:, HPC * m + h:HPC * m + h + 1],
                            scale=1.0 / np.sqrt(HD).item())
                        nc.tensor.matmul(
                            pv[0:HD + 1, :],
                            v[m][:, HW_ * h:HW_ * (h + 1)],
                            es[:],
                            start=(m == 0), stop=(m == LT - 1))
                    # colsum row (partition 64); (PV)^T rows -> ot
                    nc.vector.tensor_copy(
                        cs65[HD:HD + 1, CH * c:CH * (c + 1)], pv[HD:HD + 1, :])
                    nc.vector.tensor_copy(
                        ot[h][0:HD, CH * c:CH * (c + 1)], pv[0:HD, :])
                # this head's colsum row -> DRAM before next head reuses cs65
                nc.sync.dma_start(out=csd[h, :][None, :], in_=cs65[HD:HD + 1, :])

            # ---- softmax normalizers -----------------------------------
            # cst[p, 16h+i] = csd[h, 128i+p]   (transposing DMA per head)
            for h in range(HPC):
                dsrc = csd[h, :][None, :].rearrange("a (i q) -> a q i", q=128)
                nc.sync.dma_start(out=cst[:, 16 * h:16 * (h + 1)], in_=dsrc)
            nc.vector.reciprocal(rt[:], cst[:])
            # rscr[h, 128i+p] = rt[p, 16h+i]   (transposing DMA per head)
            for h in range(HPC):
                ddst = rscr[h, :][None, :].rearrange("a (i q) -> a q i", q=128)
                nc.sync.dma_start(out=ddst, in_=rt[:, 16 * h:16 * (h + 1)])
            for h in range(HPC):
                rb = RB.tile([64, L], F32, name="rb")
                nc.sync.dma_start(out=rb[:], in_=rscr[h, :][None, :].to_broadcast((64, L)))
                for c in range(NCH):
                    sl = ot[h][0:HD, CH * c:CH * (c + 1)]
                    nc.vector.tensor_mul(sl, sl, rb[:, CH * c:CH * (c + 1)])

            # ---- out-projection (partial over this core's 256 dims) ----
            stage_tags = ["qt0", "qt1", "kt0", "kt1"]
            for t in range(LT):
                for n in range(2):
                    ps = PSMM.tile([128, CH], F32, name="mm")
                    for k in range(HPC):
                        nc.tensor.matmul(
                            ps[:],
                            ot[k][0:HD, 128 * t:128 * (t + 1)],
                            wo_s[k][0:HD, CH * n:CH * (n + 1)],
                            start=(k == 0), stop=(k == HPC - 1))
                    idx = 2 * t + n               # 0..31
                    stage = P1.tile([128, CH], F32, name=stage_tags[idx % 4])
                    nc.scalar.copy(stage[:], ps[:])
                    nc.sync.dma_start(
                        out=out[128 * t:128 * (t + 1), CH * n:CH * (n + 1)],
                        in_=stage[:])

    nc.finalize()
    return nc


def _make_runner():
    """Compile once; return f(in_maps) -> list of per-core output dicts.

    Same execution path as concourse.bass_utils.run_bass_kernel_spmd under
    axon (bass2jax custom-call via PJRT), but with the jitted executable
    cached so repeated calls don't recompile.
    """
    import jax
    from jax.experimental.shard_map import shard_map
    from jax.sharding import Mesh, PartitionSpec
    from concourse import bass2jax, mybir

    nc = _build()
    bass2jax.install_neuronx_cc_hook()

    partition_name = nc.partition_id_tensor.name if nc.partition_id_tensor else None
    in_names, out_names, out_avals, zero_outs = [], [], [], []
    for alloc in nc.m.functions[0].allocations:
        if not isinstance(alloc, mybir.MemoryLocationSet):
            continue
        name = alloc.memorylocations[0].name
        if alloc.kind == "ExternalInput":
            if name != partition_name:
                in_names.append(name)
        elif alloc.kind == "ExternalOutput":
            out_names.append(name)
            shape = tuple(alloc.tensor_shape)
            dtype = mybir.dt.np(alloc.dtype)
            out_avals.append(jax.core.ShapedArray(shape, dtype))
            zero_outs.append(np.zeros(shape, dtype))
    n_params = len(in_names)
    n_outs = len(out_avals)
    feed_names = list(in_names) + list(out_names)
    if partition_name is not None:
        feed_names.append(partition_name)
    donate = tuple(range(n_params, n_params + n_outs))

    def _body(*args):
        operands = list(args)
        if partition_name is not None:
            operands.append(bass2jax.partition_id_tensor())
        outs = bass2jax._bass_exec_p.bind(
            *operands,
            out_avals=tuple(out_avals),
            in_names=tuple(feed_names),
            out_names=tuple(out_names),
            lowering_input_output_aliases=(),
            sim_require_finite=True,
            sim_require_nnan=True,
            nc=nc,
        )
        return tuple(outs)

    devices = jax.devices()[:NCORES]
    mesh = Mesh(np.asarray(devices), ("core",))
    sharded = jax.jit(
        shard_map(
            _body, mesh=mesh,
            in_specs=(PartitionSpec("core"),) * (n_params + n_outs),
            out_specs=(PartitionSpec("core"),) * n_outs,
            check_rep=False,
        ),
        donate_argnums=donate, keep_unused=True,
    )

    def run(in_maps):
        gi = [np.concatenate([np.asarray(m[nm]) for m in in_maps], axis=0)
              for nm in in_names]
        go = [np.concatenate([z] * NCORES, axis=0) for z in zero_outs]
        outs = sharded(*gi, *go)
        res = []
        for i in range(NCORES):
            d = {}
            for j, nm in enumerate(out_names):
                n0 = zero_outs[j].shape[0]
                d[nm] = np.asarray(outs[j][i * n0:(i + 1) * n0])
            res.append(d)
        return res

    from jax.sharding import NamedSharding
    shd = NamedSharding(mesh, PartitionSpec("core"))
    gshapes = [(NCORES * z.shape[0],) + z.shape[1:] for z in zero_outs]
    gdtypes = [z.dtype for z in zero_outs]
    make_zeros = jax.jit(
        lambda: tuple(
            jax.numpy.zeros(s, d) for s, d in zip(gshapes, gdtypes)),
        out_shardings=(shd,) * n_outs)

    def run_timed(in_maps, iters=10):
        """Device-resident repeat timing: returns list of per-iter seconds."""
        import time
        gi = [jax.device_put(
            np.concatenate([np.asarray(m[nm]) for m in in_maps], axis=0), shd)
            for nm in in_names]
        jax.block_until_ready(gi)
        ts = []
        for _ in range(iters):
            go = make_zeros()
            jax.block_until_ready(go)
            t0 = time.perf_counter()
            outs = sharded(*gi, *go)
            jax.block_until_ready(outs)
            ts.append(time.perf_counter() - t0)
        return ts

    run.timed = run_timed
    return run


def _shard_inputs(hidden_states, attention_mask, has_error_codes,
                  Wq, bq, Wk, bk, Wv, bv, Wo, bo, diag_bias, Wg, bg):
    f32 = np.float32
    hs = np.asarray(hidden_states, f32)
    am = np.asarray(attention_mask, f32).reshape(B, L)
    ec = np.asarray(has_error_codes).astype(f32)
    Wq, Wk, Wv, Wo = (np.asarray(w, f32) for w in (Wq, Wk, Wv, Wo))
    Wg = np.asarray(Wg, f32)
    bq, bk, bv, bg = (np.asarray(x, f32) for x in (bq, bk, bv, bg))
    diag = np.asarray(diag_bias, f32).reshape(NH)

    in_maps = []
    for core in range(NCORES):
        b, hb = core // 4, core % 4
        heads = range(4 * hb, 4 * hb + 4)
        cols = slice(DPC * hb, DPC * (hb + 1))
        wvg = np.zeros((H, VW), f32)
        bvg = np.zeros((VW,), f32)
        for j, h in enumerate(heads):
            wvg[:, HW_ * j:HW_ * j + HD] = Wv[:, HD * h:HD * (h + 1)]
            bvg[HW_ * j:HW_ * j + HD] = bv[HD * h:HD * (h + 1)]
            wvg[:, GW + j] = Wg[:, h]
            bvg[GW + j] = bg[h]
            bvg[HW_ * j + HD] = 1.0
        mdv = am[b][:, None] + diag[list(heads)][None, :]          # (L, 4)
        in_maps.append({
            "xT": np.ascontiguousarray(hs[b].T),
            "wq": np.ascontiguousarray(Wq[:, cols]),
            "wk": np.ascontiguousarray(Wk[:, cols]),
            "wvg": wvg,
            "wo": np.ascontiguousarray(Wo[cols, :]),
            "bq": np.ascontiguousarray(bq[cols]),
            "bk": np.ascontiguousarray(bk[cols]),
            "bvg": bvg,
            "emask": np.ascontiguousarray(ec[b].reshape(LT, 128).T),
            "md": np.ascontiguousarray(
                mdv.reshape(LT, 128, HPC).transpose(1, 0, 2).reshape(128, LT * HPC)),
        })
    return in_maps


def kernel(**inputs) -> np.ndarray:
    global _RUNNER
    if _RUNNER is None:
        _RUNNER = _make_runner()
    in_maps = _shard_inputs(**inputs)
    results = _RUNNER(in_maps)
    bo = np.asarray(inputs["bo"], np.float32)
    out = np.zeros((B, L, H), np.float32)
    for b in range(B):
        acc = np.zeros((L, H), np.float64)
        for j in range(4):
            acc += results[4 * b + j]["out"].astype(np.float64)
        out[b] = (acc + bo.astype(np.float64)).astype(np.float32)
    return out


# revision 14
# speedup vs baseline: 8111.3957x; 1.1594x over previous
"""Trainium2 Bass kernel for DiagnosticAttention (B=2, L=2048, H=1024, NH=16).

Sharding: 8 cores = 2 batches (data-parallel) x 4 head-blocks (tensor-parallel,
4 heads each).  Each core computes, for its batch b and its 4 heads:
  Q^T,K^T (head-transposed), V (+gate cols), per-head attention in S^T layout
  (keys on partitions -> all score-bias terms are per-partition, softmax sum
  via an appended ones-column in V), and a partial out-projection with its
  256 rows of Wo.  The host sums the 4 partial outputs per batch and adds bo.
"""

import sys

for _p in ("/opt/trn_rl_repo", "/root/.axon_site/_ro/trn_rl_repo"):
    if _p not in sys.path:
        sys.path.insert(0, _p)

import numpy as np

B, L, H, NH = 2, 2048, 1024, 16
HD = H // NH            # 64
NCORES = 8
HPC = 4                 # heads per core
DPC = HPC * HD          # 256 head-dims per core
KT = H // 128           # 8 contraction tiles for projections
LT = L // 128           # 16 l tiles
CH = 512                # lq chunk
NCH = L // CH           # 4 chunks
HW_ = 65                    # per-head V block: 64 V cols + ones col
GW = HPC * HW_              # 260: start of gate cols
VW = GW + HPC               # 264 total

_RUNNER = None


def _build():
    import concourse.bass as bass
    import concourse.bacc as bacc
    import concourse.tile as tile
    from concourse import mybir

    F32 = mybir.dt.float32
    BF16 = mybir.dt.bfloat16
    AL = mybir.AluOpType
    AF = mybir.ActivationFunctionType

    nc = bacc.Bacc(None, target_bir_lowering=False)

    xT = nc.dram_tensor("xT", [H, L], BF16, kind="ExternalInput")
    wq = nc.dram_tensor("wq", [H, DPC], BF16, kind="ExternalInput")
    wk = nc.dram_tensor("wk", [H, DPC], BF16, kind="ExternalInput")
    wvg = nc.dram_tensor("wvg", [H, VW], BF16, kind="ExternalInput")
    wo = nc.dram_tensor("wo", [DPC, H], BF16, kind="ExternalInput")
    bq = nc.dram_tensor("bq", [DPC], F32, kind="ExternalInput")
    bk = nc.dram_tensor("bk", [DPC], F32, kind="ExternalInput")
    bvg = nc.dram_tensor("bvg", [VW], F32, kind="ExternalInput")
    emask = nc.dram_tensor("emask", [128, LT], F32, kind="ExternalInput")
    md = nc.dram_tensor("md", [128, LT * HPC], F32, kind="ExternalInput")
    out = nc.dram_tensor("out", [L, H], F32, kind="ExternalOutput")
    csd = nc.dram_tensor("csd", [4, L], F32)
    rscr = nc.dram_tensor("rscr", [4, L], F32)

    with tile.TileContext(nc) as tc:
        with (
            tc.tile_pool(name="persist", bufs=1) as P1,
            tc.tile_pool(name="es", bufs=8) as ES,
            tc.tile_pool(name="rb", bufs=2) as RB,
            tc.tile_pool(name="psmm", bufs=2, space="PSUM") as PSMM,
            tc.tile_pool(name="pss", bufs=2, space="PSUM") as PSS,
            tc.tile_pool(name="pspv", bufs=4, space="PSUM") as PSPV,
        ):
            # ---- persistent SBUF tensors -------------------------------
            xt = [P1.tile([128, L], BF16, name=f"xt{k}") for k in range(KT)]
            wq_s = [P1.tile([128, DPC], BF16, name=f"wq{k}") for k in range(KT)]
            wk_s = [P1.tile([128, DPC], BF16, name=f"wk{k}") for k in range(KT)]
            wvg_s = [P1.tile([128, VW], BF16, name=f"wvg{k}") for k in range(KT)]
            wo_s = [P1.tile([64, H], BF16, name=f"wo{k}") for k in range(HPC)]
            qt = [P1.tile([128, L], BF16, name=f"qt{m}") for m in range(2)]
            kt = [P1.tile([128, L], BF16, name=f"kt{m}") for m in range(2)]
            v = [P1.tile([128, VW], BF16, name=f"v{t}") for t in range(LT)]
            bq_s = P1.tile([128, 2], F32, name="bqs")
            bk_s = P1.tile([128, 2], F32, name="bks")
            bvg_s = P1.tile([128, VW], F32, name="bvgs")
            em_s = P1.tile([128, LT], F32, name="ems")
            md_s = P1.tile([128, LT * HPC], F32, name="mds")
            bias_c = P1.tile([128, LT * HPC], F32, name="biasc")
            gp = P1.tile([128, HPC], F32, name="gp")
            gs = P1.tile([128, HPC], F32, name="gs")
            cst = P1.tile([128, 4 * LT], F32, name="cst")
            rt = P1.tile([128, 4 * LT], F32, name="rt")

            # ---- input DMAs --------------------------------------------
            for k in range(KT):
                nc.sync.dma_start(out=xt[k][:], in_=xT[128 * k:128 * (k + 1), :])
                nc.sync.dma_start(out=wq_s[k][:], in_=wq[128 * k:128 * (k + 1), :])
                nc.sync.dma_start(out=wk_s[k][:], in_=wk[128 * k:128 * (k + 1), :])
                nc.sync.dma_start(out=wvg_s[k][:], in_=wvg[128 * k:128 * (k + 1), :])
            for k in range(HPC):
                nc.sync.dma_start(out=wo_s[k][:], in_=wo[64 * k:64 * (k + 1), :])
            for k in range(2):
                nc.sync.dma_start(
                    out=bq_s[:, k:k + 1], in_=bq[128 * k:128 * (k + 1)][:, None])
                nc.sync.dma_start(
                    out=bk_s[:, k:k + 1], in_=bk[128 * k:128 * (k + 1)][:, None])
            nc.sync.dma_start(out=bvg_s[:], in_=bvg[None, :].to_broadcast((128, VW)))
            nc.sync.dma_start(out=em_s[:], in_=emask[:, :])
            nc.sync.dma_start(out=md_s[:], in_=md[:, :])

            # ---- Q^T / K^T projections ---------------------------------
            # qt[m][:, c] = (Wq[:, 128m:128m+128]).T @ xT chunk  + bq
            for wsrc, bsrc, dst in ((wq_s, bq_s, qt), (wk_s, bk_s, kt)):
                for m in range(2):
                    for c in range(NCH):
                        ps = PSMM.tile([128, CH], F32, name="mm")
                        for k in range(KT):
                            nc.tensor.matmul(
                                ps[:],
                                wsrc[k][:, 128 * m:128 * (m + 1)],
                                xt[k][:, CH * c:CH * (c + 1)],
                                start=(k == 0), stop=(k == KT - 1))
                        nc.vector.tensor_scalar_add(
                            dst[m][:, CH * c:CH * (c + 1)], ps[:],
                            bsrc[:, m:m + 1])

            # ---- V (+gate) projection, bias cols -----------------------
            for t in range(LT):
                ps = PSMM.tile([128, CH], F32, name="mm")
                pv = ps[:, 0:VW]
                for k in range(KT):
                    nc.tensor.matmul(
                        pv,
                        xt[k][:, 128 * t:128 * (t + 1)],
                        wvg_s[k][:],
                        start=(k == 0), stop=(k == KT - 1))
                # gate: sigmoid(x@Wg + bg), then bias col = em*gate + md
                nc.vector.tensor_add(
                    gp[:], ps[:, GW:VW], bvg_s[:, GW:VW])
                nc.scalar.activation(gs[:], gp[:], AF.Sigmoid)
                nc.vector.scalar_tensor_tensor(
                    out=bias_c[:, HPC * t:HPC * (t + 1)],
                    in0=gs[:], scalar=em_s[:, t:t + 1],
                    in1=md_s[:, HPC * t:HPC * (t + 1)],
                    op0=AL.mult, op1=AL.add)
                # V(+bias) -> SBUF, then ones column per head
                nc.vector.tensor_add(
                    v[t][:, 0:GW], pv[:, 0:GW], bvg_s[:, 0:GW])
                # ones columns come via bvg (host sets bvg[65h+64]=1)

            # ---- attention, S^T layout ---------------------------------
            # (PV)^T accumulators in f32 (normalized in f32, cast to bf16 later)
            ot = [P1.tile([64, L], F32, name=f"ot{h}") for h in range(HPC)]
            cs2 = P1.tile([65, 2 * L], F32, name="cs2")
            for hp in range(2):
                ha, hb = 2 * hp, 2 * hp + 1
                for c in range(NCH):
                    pva = PSPV.tile([128, CH], F32, name="pv")
                    pvb = PSPV.tile([128, CH], F32, name="pv")
                    for m in range(LT):
                        ess = []
                        for h in (ha, hb):
                            hf = 64 * (h % 2)
                            ss = PSS.tile([128, CH], F32, name="ss")
                            nc.tensor.matmul(
                                ss[:],
                                kt[hp][hf:hf + 64, 128 * m:128 * (m + 1)],
                                qt[hp][hf:hf + 64, CH * c:CH * (c + 1)],
                                start=True, stop=True)
                            es = ES.tile([128, CH], BF16, name="es")
                            nc.scalar.activation(
                                es[:], ss[:], AF.Exp,
                                bias=bias_c[:, HPC * m + h:HPC * m + h + 1],
                                scale=1.0 / np.sqrt(HD).item())
                            ess.append(es)
                        for h, pv, es in ((ha, pva, ess[0]), (hb, pvb, ess[1])):
                            nc.tensor.matmul(
                                pv[0:HD + 1, :],
                                v[m][:, HW_ * h:HW_ * (h + 1)],
                                es[:],
                                start=(m == 0), stop=(m == LT - 1))
                    for h, pv in ((ha, pva), (hb, pvb)):
                        # colsum rows parked on partition 64, head-major
                        nc.vector.tensor_copy(
                            cs2[HD:HD + 1, L * (h % 2) + CH * c:
                                L * (h % 2) + CH * (c + 1)],
                            pv[HD:HD + 1, :])
                        nc.vector.tensor_copy(
                            ot[h][0:HD, CH * c:CH * (c + 1)], pv[0:HD, :])
                # both heads' colsum rows -> DRAM before next pair reuses cs2
                nc.sync.dma_start(out=csd[ha, :][None, :], in_=cs2[HD:HD + 1, 0:L])
                nc.sync.dma_start(out=csd[hb, :][None, :], in_=cs2[HD:HD + 1, L:2 * L])

            # ---- softmax normalizers -----------------------------------
            # cst[p, 16h+i] = csd[h, 128i+p]   (transposing DMA per head)
            for h in range(HPC):
                dsrc = csd[h, :][None, :].rearrange("a (i q) -> a q i", q=128)
                nc.sync.dma_start(out=cst[:, 16 * h:16 * (h + 1)], in_=dsrc)
            nc.vector.reciprocal(rt[:], cst[:])
            # rscr[h, 128i+p] = rt[p, 16h+i]   (transposing DMA per head)
            for h in range(HPC):
                ddst = rscr[h, :][None, :].rearrange("a (i q) -> a q i", q=128)
                nc.sync.dma_start(out=ddst, in_=rt[:, 16 * h:16 * (h + 1)])
            otb = [P1.tile([64, L], BF16, name=f"xt{h}") for h in range(HPC)]
            for h in range(HPC):
                rb = RB.tile([64, L], F32, name="rb")
                nc.sync.dma_start(out=rb[:], in_=rscr[h, :][None, :].to_broadcast((64, L)))
                for c in range(NCH):
                    sl = ot[h][0:HD, CH * c:CH * (c + 1)]
                    nc.vector.tensor_mul(
                        otb[h][0:HD, CH * c:CH * (c + 1)],
                        sl, rb[:, CH * c:CH * (c + 1)])

            # ---- out-projection (partial over this core's 256 dims) ----
            stage_tags = ["qt0", "qt1", "kt0", "kt1"]
            for t in range(LT):
                for n in range(2):
                    ps = PSMM.tile([128, CH], F32, name="mm")
                    for k in range(HPC):
                        nc.tensor.matmul(
                            ps[:],
                            otb[k][0:HD, 128 * t:128 * (t + 1)],
                            wo_s[k][0:HD, CH * n:CH * (n + 1)],
                            start=(k == 0), stop=(k == HPC - 1))
                    idx = 2 * t + n               # 0..31
                    stage = P1.tile([128, CH], F32, name=stage_tags[idx % 4])
                    nc.vector.tensor_copy(stage[:], ps[:])
                    nc.sync.dma_start(
                        out=out[128 * t:128 * (t + 1), CH * n:CH * (n + 1)],
                        in_=stage[:])

    nc.finalize()
    return nc


def _make_runner():
    """Compile once; return f(in_maps) -> list of per-core output dicts.

    Same execution path as concourse.bass_utils.run_bass_kernel_spmd under
    axon (bass2jax custom-call via PJRT), but with the jitted executable
    cached so repeated calls don't recompile.
    """
    import jax
    from jax.experimental.shard_map import shard_map
    from jax.sharding import Mesh, PartitionSpec
    from concourse import bass2jax, mybir

    nc = _build()
    bass2jax.install_neuronx_cc_hook()

    partition_name = nc.partition_id_tensor.name if nc.partition_id_tensor else None
    in_names, out_names, out_avals, zero_outs = [], [], [], []
    for alloc in nc.m.functions[0].allocations:
        if not isinstance(alloc, mybir.MemoryLocationSet):
            continue
        name = alloc.memorylocations[0].name
        if alloc.kind == "ExternalInput":
            if name != partition_name:
                in_names.append(name)
        elif alloc.kind == "ExternalOutput":
            out_names.append(name)
            shape = tuple(alloc.tensor_shape)
            dtype = mybir.dt.np(alloc.dtype)
            out_avals.append(jax.core.ShapedArray(shape, dtype))
            zero_outs.append(np.zeros(shape, dtype))
    n_params = len(in_names)
    n_outs = len(out_avals)
    feed_names = list(in_names) + list(out_names)
    if partition_name is not None:
        feed_names.append(partition_name)
    donate = tuple(range(n_params, n_params + n_outs))

    def _body(*args):
        operands = list(args)
        if partition_name is not None:
            operands.append(bass2jax.partition_id_tensor())
        outs = bass2jax._bass_exec_p.bind(
            *operands,
            out_avals=tuple(out_avals),
            in_names=tuple(feed_names),
            out_names=tuple(out_names),
            lowering_input_output_aliases=(),
            sim_require_finite=True,
            sim_require_nnan=True,
            nc=nc,
        )
        return tuple(outs)

    devices = jax.devices()[:NCORES]
    mesh = Mesh(np.asarray(devices), ("core",))
    sharded = jax.jit(
        shard_map(
            _body, mesh=mesh,
            in_specs=(PartitionSpec("core"),) * (n_params + n_outs),
            out_specs=(PartitionSpec("core"),) * n_outs,
            check_rep=False,
        ),
        donate_argnums=donate, keep_unused=True,
    )

    def run(in_maps):
        gi = [np.concatenate([np.asarray(m[nm]) for m in in_maps], axis=0)
              for nm in in_names]
        go = [np.concatenate([z] * NCORES, axis=0) for z in zero_outs]
        outs = sharded(*gi, *go)
        res = []
        for i in range(NCORES):
            d = {}
            for j, nm in enumerate(out_names):
                n0 = zero_outs[j].shape[0]
                d[nm] = np.asarray(outs[j][i * n0:(i + 1) * n0])
            res.append(d)
        return res

    from jax.sharding import NamedSharding
    shd = NamedSharding(mesh, PartitionSpec("core"))
    gshapes = [(NCORES * z.shape[0],) + z.shape[1:] for z in zero_outs]
    gdtypes = [z.dtype for z in zero_outs]
    make_zeros = jax.jit(
        lambda: tuple(
            jax.numpy.zeros(s, d) for s, d in zip(gshapes, gdtypes)),
        out_shardings=(shd,) * n_outs)

    def run_timed(in_maps, iters=10):
        """Device-resident repeat timing: returns list of per-iter seconds."""
        import time
        gi = [jax.device_put(
            np.concatenate([np.asarray(m[nm]) for m in in_maps], axis=0), shd)
            for nm in in_names]
        jax.block_until_ready(gi)
        ts = []
        for _ in range(iters):
            go = make_zeros()
            jax.block_until_ready(go)
            t0 = time.perf_counter()
            outs = sharded(*gi, *go)
            jax.block_until_ready(outs)
            ts.append(time.perf_counter() - t0)
        return ts

    run.timed = run_timed
    return run


def _shard_inputs(hidden_states, attention_mask, has_error_codes,
                  Wq, bq, Wk, bk, Wv, bv, Wo, bo, diag_bias, Wg, bg):
    import ml_dtypes
    bf16 = ml_dtypes.bfloat16
    f32 = np.float32
    hs = np.asarray(hidden_states, f32)
    am = np.asarray(attention_mask, f32).reshape(B, L)
    ec = np.asarray(has_error_codes).astype(f32)
    Wq, Wk, Wv, Wo = (np.asarray(w, f32) for w in (Wq, Wk, Wv, Wo))
    Wg = np.asarray(Wg, f32)
    bq, bk, bv, bg = (np.asarray(x, f32) for x in (bq, bk, bv, bg))
    diag = np.asarray(diag_bias, f32).reshape(NH)

    in_maps = []
    for core in range(NCORES):
        b, hb = core // 4, core % 4
        heads = range(4 * hb, 4 * hb + 4)
        cols = slice(DPC * hb, DPC * (hb + 1))
        wvg = np.zeros((H, VW), f32)
        bvg = np.zeros((VW,), f32)
        for j, h in enumerate(heads):
            wvg[:, HW_ * j:HW_ * j + HD] = Wv[:, HD * h:HD * (h + 1)]
            bvg[HW_ * j:HW_ * j + HD] = bv[HD * h:HD * (h + 1)]
            wvg[:, GW + j] = Wg[:, h]
            bvg[GW + j] = bg[h]
            bvg[HW_ * j + HD] = 1.0
        mdv = am[b][:, None] + diag[list(heads)][None, :]          # (L, 4)
        in_maps.append({
            "xT": np.ascontiguousarray(hs[b].T).astype(bf16),
            "wq": np.ascontiguousarray(Wq[:, cols]).astype(bf16),
            "wk": np.ascontiguousarray(Wk[:, cols]).astype(bf16),
            "wvg": wvg.astype(bf16),
            "wo": np.ascontiguousarray(Wo[cols, :]).astype(bf16),
            "bq": np.ascontiguousarray(bq[cols]),
            "bk": np.ascontiguousarray(bk[cols]),
            "bvg": bvg,
            "emask": np.ascontiguousarray(ec[b].reshape(LT, 128).T),
            "md": np.ascontiguousarray(
                mdv.reshape(LT, 128, HPC).transpose(1, 0, 2).reshape(128, LT * HPC)),
        })
    return in_maps


def kernel(**inputs) -> np.ndarray:
    global _RUNNER
    if _RUNNER is None:
        _RUNNER = _make_runner()
    in_maps = _shard_inputs(**inputs)
    results = _RUNNER(in_maps)
    bo = np.asarray(inputs["bo"], np.float32)
    out = np.zeros((B, L, H), np.float32)
    for b in range(B):
        acc = np.zeros((L, H), np.float64)
        for j in range(4):
            acc += results[4 * b + j]["out"].astype(np.float64)
        out[b] = (acc + bo.astype(np.float64)).astype(np.float32)
    return out


# revision 16
# speedup vs baseline: 11513.5699x; 1.4194x over previous
"""Trainium2 Bass kernel for DiagnosticAttention (B=2, L=2048, H=1024, NH=16).

Sharding: 8 cores = 2 batches (data-parallel) x 4 head-blocks (tensor-parallel,
4 heads each).  Each core computes, for its batch b and its 4 heads:
  Q^T,K^T (head-transposed), V (+gate cols), per-head attention in S^T layout
  (keys on partitions -> all score-bias terms are per-partition, softmax sum
  via an appended ones-column in V), and a partial out-projection with its
  256 rows of Wo.  The host sums the 4 partial outputs per batch and adds bo.
"""

import sys

for _p in ("/opt/trn_rl_repo", "/root/.axon_site/_ro/trn_rl_repo"):
    if _p not in sys.path:
        sys.path.insert(0, _p)

import numpy as np

B, L, H, NH = 2, 2048, 1024, 16
HD = H // NH            # 64
NCORES = 8
HPC = 4                 # heads per core
DPC = HPC * HD          # 256 head-dims per core
KT = H // 128           # 8 contraction tiles for projections
LT = L // 128           # 16 l tiles
CH = 512                # lq chunk
NCH = L // CH           # 4 chunks
HW_ = 65                    # per-head V block: 64 V cols + ones col
GW = HPC * HW_              # 260: start of gate cols
VW = GW + HPC               # 264 total

_RUNNER = None


def _build():
    import concourse.bass as bass
    import concourse.bacc as bacc
    import concourse.tile as tile
    from concourse import mybir

    F32 = mybir.dt.float32
    BF16 = mybir.dt.bfloat16
    AL = mybir.AluOpType
    AF = mybir.ActivationFunctionType

    nc = bacc.Bacc(None, target_bir_lowering=False)

    xT = nc.dram_tensor("xT", [H, L], BF16, kind="ExternalInput")
    wq = nc.dram_tensor("wq", [H, DPC], BF16, kind="ExternalInput")
    wk = nc.dram_tensor("wk", [H, DPC], BF16, kind="ExternalInput")
    wvg = nc.dram_tensor("wvg", [H, VW], BF16, kind="ExternalInput")
    wo = nc.dram_tensor("wo", [DPC, H], BF16, kind="ExternalInput")
    bq = nc.dram_tensor("bq", [DPC], F32, kind="ExternalInput")
    bk = nc.dram_tensor("bk", [DPC], F32, kind="ExternalInput")
    bvg = nc.dram_tensor("bvg", [VW], F32, kind="ExternalInput")
    emask = nc.dram_tensor("emask", [128, LT], F32, kind="ExternalInput")
    md = nc.dram_tensor("md", [128, LT * HPC], F32, kind="ExternalInput")
    out = nc.dram_tensor("out", [L, H], F32, kind="ExternalOutput")
    csd = nc.dram_tensor("csd", [4, L], F32)
    rscr = nc.dram_tensor("rscr", [4, L], F32)

    with tile.TileContext(nc) as tc:
        with (
            tc.tile_pool(name="persist", bufs=1) as P1,
            tc.tile_pool(name="es", bufs=6) as ES,
            tc.tile_pool(name="rb", bufs=2) as RB,
            tc.tile_pool(name="ps", bufs=2, space="PSUM") as PS,
        ):
            # ---- persistent SBUF tensors -------------------------------
            xt = [P1.tile([128, L], BF16, name=f"xt{k}") for k in range(KT)]
            wq_s = [P1.tile([128, DPC], BF16, name=f"wq{k}") for k in range(KT)]
            wk_s = [P1.tile([128, DPC], BF16, name=f"wk{k}") for k in range(KT)]
            wvg_s = [P1.tile([128, VW], BF16, name=f"wvg{k}") for k in range(KT)]
            wo_s = [P1.tile([64, H], BF16, name=f"wo{k}") for k in range(HPC)]
            qt = [P1.tile([128, L], BF16, name=f"qt{m}") for m in range(2)]
            kt = [P1.tile([128, L], BF16, name=f"kt{m}") for m in range(2)]
            v = [P1.tile([128, VW], BF16, name=f"v{t}") for t in range(LT)]
            bq_s = P1.tile([128, 2], F32, name="bqs")
            bk_s = P1.tile([128, 2], F32, name="bks")
            bvg_s = P1.tile([128, VW], F32, name="bvgs")
            em_s = P1.tile([128, LT], F32, name="ems")
            md_s = P1.tile([128, LT * HPC], F32, name="mds")
            bias_c = P1.tile([128, LT * HPC], F32, name="biasc")
            gp = P1.tile([128, HPC], F32, name="gp")
            gs = P1.tile([128, HPC], F32, name="gs")
            cst = P1.tile([128, 4 * LT], F32, name="cst")
            rt = P1.tile([128, 4 * LT], F32, name="rt")

            # ---- input DMAs --------------------------------------------
            for k in range(KT):
                nc.sync.dma_start(out=xt[k][:], in_=xT[128 * k:128 * (k + 1), :])
                nc.sync.dma_start(out=wq_s[k][:], in_=wq[128 * k:128 * (k + 1), :])
                nc.sync.dma_start(out=wk_s[k][:], in_=wk[128 * k:128 * (k + 1), :])
                nc.sync.dma_start(out=wvg_s[k][:], in_=wvg[128 * k:128 * (k + 1), :])
            for k in range(HPC):
                nc.sync.dma_start(out=wo_s[k][:], in_=wo[64 * k:64 * (k + 1), :])
            for k in range(2):
                nc.sync.dma_start(
                    out=bq_s[:, k:k + 1], in_=bq[128 * k:128 * (k + 1)][:, None])
                nc.sync.dma_start(
                    out=bk_s[:, k:k + 1], in_=bk[128 * k:128 * (k + 1)][:, None])
            nc.sync.dma_start(out=bvg_s[:], in_=bvg[None, :].to_broadcast((128, VW)))
            nc.sync.dma_start(out=em_s[:], in_=emask[:, :])
            nc.sync.dma_start(out=md_s[:], in_=md[:, :])

            # ---- Q^T / K^T projections ---------------------------------
            # qt[m][:, c] = (Wq[:, 128m:128m+128]).T @ xT chunk  + bq
            for wsrc, bsrc, dst in ((wq_s, bq_s, qt), (wk_s, bk_s, kt)):
                for m in range(2):
                    for c in range(NCH):
                        ps = PS.tile([128, CH], F32, name="mm", tag="ss")
                        for k in range(KT):
                            nc.tensor.matmul(
                                ps[:],
                                wsrc[k][:, 128 * m:128 * (m + 1)],
                                xt[k][:, CH * c:CH * (c + 1)],
                                start=(k == 0), stop=(k == KT - 1))
                        nc.vector.tensor_scalar_add(
                            dst[m][:, CH * c:CH * (c + 1)], ps[:],
                            bsrc[:, m:m + 1])

            # ---- V (+gate) projection, bias cols -----------------------
            for t in range(LT):
                ps = PS.tile([128, CH], F32, name="mm", tag="ss")
                pv = ps[:, 0:VW]
                for k in range(KT):
                    nc.tensor.matmul(
                        pv,
                        xt[k][:, 128 * t:128 * (t + 1)],
                        wvg_s[k][:],
                        start=(k == 0), stop=(k == KT - 1))
                # gate: sigmoid(x@Wg + bg), then bias col = em*gate + md
                nc.vector.tensor_add(
                    gp[:], ps[:, GW:VW], bvg_s[:, GW:VW])
                nc.scalar.activation(gs[:], gp[:], AF.Exp, scale=-1.0)
                nc.vector.tensor_scalar_add(gs[:], gs[:], 1.0)
                nc.vector.reciprocal(gs[:], gs[:])
                nc.vector.scalar_tensor_tensor(
                    out=bias_c[:, HPC * t:HPC * (t + 1)],
                    in0=gs[:], scalar=em_s[:, t:t + 1],
                    in1=md_s[:, HPC * t:HPC * (t + 1)],
                    op0=AL.mult, op1=AL.add)
                # V(+bias) -> SBUF, then ones column per head
                nc.vector.tensor_add(
                    v[t][:, 0:GW], pv[:, 0:GW], bvg_s[:, 0:GW])
                # ones columns come via bvg (host sets bvg[65h+64]=1)

            # ---- attention, S^T layout ---------------------------------
            # (PV)^T accumulators in f32 (normalized in f32, cast to bf16 later)
            ot = [P1.tile([64, L], F32, name=f"ot{h}") for h in range(HPC)]
            cs2 = P1.tile([65, 2 * L], F32, name="cs2")
            SC = (1.0 / np.sqrt(HD)).item() if hasattr(np.sqrt(HD), 'item') else 1.0 / float(np.sqrt(HD))
            for hp in range(2):
                ha, hb = 2 * hp, 2 * hp + 1
                for cp in range(NCH // 2):
                    c0 = 2 * cp
                    pvs = {}
                    for h in (ha, hb):
                        for j in range(2):
                            pvs[(h, j)] = PS.tile(
                                [128, CH], F32, name="pv", tag="pv", bufs=4)
                    for m in range(LT):
                        for h in (ha, hb):
                            hf = 64 * (h % 2)
                            ss2 = PS.tile([128, 2 * CH], F32, name="ss2", tag="ss")
                            for j in range(2):
                                nc.tensor.matmul(
                                    ss2[:, CH * j:CH * (j + 1)],
                                    kt[hp][hf:hf + 64, 128 * m:128 * (m + 1)],
                                    qt[hp][hf:hf + 64,
                                           CH * (c0 + j):CH * (c0 + j + 1)],
                                    start=True, stop=True)
                            es2 = ES.tile([128, 2 * CH], BF16, name="es")
                            nc.scalar.activation(
                                es2[:], ss2[:], AF.Exp,
                                bias=bias_c[:, HPC * m + h:HPC * m + h + 1],
                                scale=SC)
                            for j in range(2):
                                nc.tensor.matmul(
                                    pvs[(h, j)][0:HD + 1, :],
                                    v[m][:, HW_ * h:HW_ * (h + 1)],
                                    es2[:, CH * j:CH * (j + 1)],
                                    start=(m == 0), stop=(m == LT - 1))
                    for h in (ha, hb):
                        for j in range(2):
                            pv = pvs[(h, j)]
                            cc = c0 + j
                            nc.vector.tensor_copy(
                                cs2[HD:HD + 1, L * (h % 2) + CH * cc:
                                    L * (h % 2) + CH * (cc + 1)],
                                pv[HD:HD + 1, :])
                            nc.vector.tensor_copy(
                                ot[h][0:HD, CH * cc:CH * (cc + 1)], pv[0:HD, :])
                # both heads' colsum rows -> DRAM before next pair reuses cs2
                nc.sync.dma_start(out=csd[ha, :][None, :], in_=cs2[HD:HD + 1, 0:L])
                nc.sync.dma_start(out=csd[hb, :][None, :], in_=cs2[HD:HD + 1, L:2 * L])

            # ---- softmax normalizers -----------------------------------
            # cst[p, 16h+i] = csd[h, 128i+p]   (transposing DMA per head)
            for h in range(HPC):
                dsrc = csd[h, :][None, :].rearrange("a (i q) -> a q i", q=128)
                nc.sync.dma_start(out=cst[:, 16 * h:16 * (h + 1)], in_=dsrc)
            nc.vector.reciprocal(rt[:], cst[:])
            # rscr[h, 128i+p] = rt[p, 16h+i]   (transposing DMA per head)
            for h in range(HPC):
                ddst = rscr[h, :][None, :].rearrange("a (i q) -> a q i", q=128)
                nc.sync.dma_start(out=ddst, in_=rt[:, 16 * h:16 * (h + 1)])
            otb = [P1.tile([64, L], BF16, name=f"xt{h}") for h in range(HPC)]
            for h in range(HPC):
                rb = RB.tile([64, L], F32, name="rb")
                nc.sync.dma_start(out=rb[:], in_=rscr[h, :][None, :].to_broadcast((64, L)))
                for c in range(NCH):
                    sl = ot[h][0:HD, CH * c:CH * (c + 1)]
                    nc.vector.tensor_mul(
                        otb[h][0:HD, CH * c:CH * (c + 1)],
                        sl, rb[:, CH * c:CH * (c + 1)])

            # ---- out-projection (partial over this core's 256 dims) ----
            stage_tags = ["qt0", "qt1", "kt0", "kt1"]
            for t in range(LT):
                for n in range(2):
                    ps = PS.tile([128, CH], F32, name="mm", tag="ss")
                    for k in range(HPC):
                        nc.tensor.matmul(
                            ps[:],
                            otb[k][0:HD, 128 * t:128 * (t + 1)],
                            wo_s[k][0:HD, CH * n:CH * (n + 1)],
                            start=(k == 0), stop=(k == HPC - 1))
                    idx = 2 * t + n               # 0..31
                    stage = P1.tile([128, CH], F32, name=stage_tags[idx % 4])
                    nc.vector.tensor_copy(stage[:], ps[:])
                    nc.sync.dma_start(
                        out=out[128 * t:128 * (t + 1), CH * n:CH * (n + 1)],
                        in_=stage[:])

    nc.finalize()
    return nc


def _make_runner():
    """Compile once; return f(in_maps) -> list of per-core output dicts.

    Same execution path as concourse.bass_utils.run_bass_kernel_spmd under
    axon (bass2jax custom-call via PJRT), but with the jitted executable
    cached so repeated calls don't recompile.
    """
    import jax
    from jax.experimental.shard_map import shard_map
    from jax.sharding import Mesh, PartitionSpec
    from concourse import bass2jax, mybir

    nc = _build()
    bass2jax.install_neuronx_cc_hook()

    partition_name = nc.partition_id_tensor.name if nc.partition_id_tensor else None
    in_names, out_names, out_avals, zero_outs = [], [], [], []
    for alloc in nc.m.functions[0].allocations:
        if not isinstance(alloc, mybir.MemoryLocationSet):
            continue
        name = alloc.memorylocations[0].name
        if alloc.kind == "ExternalInput":
            if name != partition_name:
                in_names.append(name)
        elif alloc.kind == "ExternalOutput":
            out_names.append(name)
            shape = tuple(alloc.tensor_shape)
            dtype = mybir.dt.np(alloc.dtype)
            out_avals.append(jax.core.ShapedArray(shape, dtype))
            zero_outs.append(np.zeros(shape, dtype))
    n_params = len(in_names)
    n_outs = len(out_avals)
    feed_names = list(in_names) + list(out_names)
    if partition_name is not None:
        feed_names.append(partition_name)
    donate = tuple(range(n_params, n_params + n_outs))

    def _body(*args):
        operands = list(args)
        if partition_name is not None:
            operands.append(bass2jax.partition_id_tensor())
        outs = bass2jax._bass_exec_p.bind(
            *operands,
            out_avals=tuple(out_avals),
            in_names=tuple(feed_names),
            out_names=tuple(out_names),
            lowering_input_output_aliases=(),
            sim_require_finite=True,
            sim_require_nnan=True,
            nc=nc,
        )
        return tuple(outs)

    devices = jax.devices()[:NCORES]
    mesh = Mesh(np.asarray(devices), ("core",))
    sharded = jax.jit(
        shard_map(
            _body, mesh=mesh,
            in_specs=(PartitionSpec("core"),) * (n_params + n_outs),
            out_specs=(PartitionSpec("core"),) * n_outs,
            check_rep=False,
        ),
        donate_argnums=donate, keep_unused=True,
    )

    def run(in_maps):
        gi = [np.concatenate([np.asarray(m[nm]) for m in in_maps], axis=0)
              for nm in in_names]
        go = [np.concatenate([z] * NCORES, axis=0) for z in zero_outs]
        outs = sharded(*gi, *go)
        res = []
        for i in range(NCORES):
            d = {}
            for j, nm in enumerate(out_names):
                n0 = zero_outs[j].shape[0]
                d[nm] = np.asarray(outs[j][i * n0:(i + 1) * n0])
            res.append(d)
        return res

    from jax.sharding import NamedSharding
    shd = NamedSharding(mesh, PartitionSpec("core"))
    gshapes = [(NCORES * z.shape[0],) + z.shape[1:] for z in zero_outs]
    gdtypes = [z.dtype for z in zero_outs]
    make_zeros = jax.jit(
        lambda: tuple(
            jax.numpy.zeros(s, d) for s, d in zip(gshapes, gdtypes)),
        out_shardings=(shd,) * n_outs)

    def run_timed(in_maps, iters=10):
        """Device-resident repeat timing: returns list of per-iter seconds."""
        import time
        gi = [jax.device_put(
            np.concatenate([np.asarray(m[nm]) for m in in_maps], axis=0), shd)
            for nm in in_names]
        jax.block_until_ready(gi)
        ts = []
        for _ in range(iters):
            go = make_zeros()
            jax.block_until_ready(go)
            t0 = time.perf_counter()
            outs = sharded(*gi, *go)
            jax.block_until_ready(outs)
            ts.append(time.perf_counter() - t0)
        return ts

    run.timed = run_timed
    return run


def _shard_inputs(hidden_states, attention_mask, has_error_codes,
                  Wq, bq, Wk, bk, Wv, bv, Wo, bo, diag_bias, Wg, bg):
    import ml_dtypes
    bf16 = ml_dtypes.bfloat16
    f32 = np.float32
    hs = np.asarray(hidden_states, f32)
    am = np.asarray(attention_mask, f32).reshape(B, L)
    ec = np.asarray(has_error_codes).astype(f32)
    Wq, Wk, Wv, Wo = (np.asarray(w, f32) for w in (Wq, Wk, Wv, Wo))
    Wg = np.asarray(Wg, f32)
    bq, bk, bv, bg = (np.asarray(x, f32) for x in (bq, bk, bv, bg))
    diag = np.asarray(diag_bias, f32).reshape(NH)

    in_maps = []
    for core in range(NCORES):
        b, hb = core // 4, core % 4
        heads = range(4 * hb, 4 * hb + 4)
        cols = slice(DPC * hb, DPC * (hb + 1))
        wvg = np.zeros((H, VW), f32)
        bvg = np.zeros((VW,), f32)
        for j, h in enumerate(heads):
            wvg[:, HW_ * j:HW_ * j + HD] = Wv[:, HD * h:HD * (h + 1)]
            bvg[HW_ * j:HW_ * j + HD] = bv[HD * h:HD * (h + 1)]
            wvg[:, GW + j] = Wg[:, h]
            bvg[GW + j] = bg[h]
            bvg[HW_ * j + HD] = 1.0
        mdv = am[b][:, None] + diag[list(heads)][None, :]          # (L, 4)
        in_maps.append({
            "xT": np.ascontiguousarray(hs[b].T).astype(bf16),
            "wq": np.ascontiguousarray(Wq[:, cols]).astype(bf16),
            "wk": np.ascontiguousarray(Wk[:, cols]).astype(bf16),
            "wvg": wvg.astype(bf16),
            "wo": np.ascontiguousarray(Wo[cols, :]).astype(bf16),
            "bq": np.ascontiguousarray(bq[cols]),
            "bk": np.ascontiguousarray(bk[cols]),
            "bvg": bvg,
            "emask": np.ascontiguousarray(ec[b].reshape(LT, 128).T),
            "md": np.ascontiguousarray(
                mdv.reshape(LT, 128, HPC).transpose(1, 0, 2).reshape(128, LT * HPC)),
        })
    return in_maps


def kernel(**inputs) -> np.ndarray:
    global _RUNNER
    if _RUNNER is None:
        _RUNNER = _make_runner()
    in_maps = _shard_inputs(**inputs)
    results = _RUNNER(in_maps)
    bo = np.asarray(inputs["bo"], np.float32)
    out = np.zeros((B, L, H), np.float32)
    for b in range(B):
        acc = np.zeros((L, H), np.float64)
        for j in range(4):
            acc += results[4 * b + j]["out"].astype(np.float64)
        out[b] = (acc + bo.astype(np.float64)).astype(np.float32)
    return out


# revision 17
# speedup vs baseline: 11520.1731x; 1.0006x over previous
"""Trainium2 Bass kernel for DiagnosticAttention (B=2, L=2048, H=1024, NH=16).

Sharding: 8 cores = 2 batches (data-parallel) x 4 head-blocks (tensor-parallel,
4 heads each).  Each core computes, for its batch b and its 4 heads:
  Q^T,K^T (head-transposed), V (+gate cols), per-head attention in S^T layout
  (keys on partitions -> all score-bias terms are per-partition, softmax sum
  via an appended ones-column in V), and a partial out-projection with its
  256 rows of Wo.  The host sums the 4 partial outputs per batch and adds bo.
"""

import sys

for _p in ("/opt/trn_rl_repo", "/root/.axon_site/_ro/trn_rl_repo"):
    if _p not in sys.path:
        sys.path.insert(0, _p)

import numpy as np

B, L, H, NH = 2, 2048, 1024, 16
HD = H // NH            # 64
NCORES = 8
HPC = 4                 # heads per core
DPC = HPC * HD          # 256 head-dims per core
KT = H // 128           # 8 contraction tiles for projections
LT = L // 128           # 16 l tiles
CH = 512                # lq chunk
NCH = L // CH           # 4 chunks
HW_ = 65                    # per-head V block: 64 V cols + ones col
GW = HPC * HW_              # 260: start of gate cols
VW = GW + HPC               # 264 total

_RUNNER = None


def _build():
    import concourse.bass as bass
    import concourse.bacc as bacc
    import concourse.tile as tile
    from concourse import mybir

    F32 = mybir.dt.float32
    BF16 = mybir.dt.bfloat16
    AL = mybir.AluOpType
    AF = mybir.ActivationFunctionType

    nc = bacc.Bacc(None, target_bir_lowering=False)

    xT = nc.dram_tensor("xT", [H, L], BF16, kind="ExternalInput")
    wq = nc.dram_tensor("wq", [H, DPC], BF16, kind="ExternalInput")
    wk = nc.dram_tensor("wk", [H, DPC], BF16, kind="ExternalInput")
    wvg = nc.dram_tensor("wvg", [H, VW], BF16, kind="ExternalInput")
    wo = nc.dram_tensor("wo", [DPC, H], BF16, kind="ExternalInput")
    bq = nc.dram_tensor("bq", [DPC], F32, kind="ExternalInput")
    bk = nc.dram_tensor("bk", [DPC], F32, kind="ExternalInput")
    bvg = nc.dram_tensor("bvg", [VW], F32, kind="ExternalInput")
    emask = nc.dram_tensor("emask", [128, LT], F32, kind="ExternalInput")
    md = nc.dram_tensor("md", [128, LT * HPC], F32, kind="ExternalInput")
    out = nc.dram_tensor("out", [L, H], F32, kind="ExternalOutput")
    csd = nc.dram_tensor("csd", [4, L], F32)
    rscr = nc.dram_tensor("rscr", [4, L], F32)

    with tile.TileContext(nc) as tc:
        with (
            tc.tile_pool(name="persist", bufs=1) as P1,
            tc.tile_pool(name="es", bufs=6) as ES,
            tc.tile_pool(name="rb", bufs=2) as RB,
            tc.tile_pool(name="ps", bufs=2, space="PSUM") as PS,
        ):
            # ---- persistent SBUF tensors -------------------------------
            xt = [P1.tile([128, L], BF16, name=f"xt{k}") for k in range(KT)]
            wq_s = [P1.tile([128, DPC], BF16, name=f"wq{k}") for k in range(KT)]
            wk_s = [P1.tile([128, DPC], BF16, name=f"wk{k}") for k in range(KT)]
            wvg_s = [P1.tile([128, VW], BF16, name=f"wvg{k}") for k in range(KT)]
            wo_s = [P1.tile([64, H], BF16, name=f"wo{k}") for k in range(HPC)]
            qt = [P1.tile([128, L], BF16, name=f"qt{m}") for m in range(2)]
            kt = [P1.tile([128, L], BF16, name=f"kt{m}") for m in range(2)]
            v = [P1.tile([128, VW], BF16, name=f"v{t}") for t in range(LT)]
            bq_s = P1.tile([128, 2], F32, name="bqs")
            bk_s = P1.tile([128, 2], F32, name="bks")
            bvg_s = P1.tile([128, VW], F32, name="bvgs")
            em_s = P1.tile([128, LT], F32, name="ems")
            md_s = P1.tile([128, LT * HPC], F32, name="mds")
            bias_c = P1.tile([128, LT * HPC], F32, name="biasc")
            gp = P1.tile([128, HPC], F32, name="gp")
            gs = P1.tile([128, HPC], F32, name="gs")
            cst = P1.tile([128, 4 * LT], F32, name="cst")
            rt = P1.tile([128, 4 * LT], F32, name="rt")

            # ---- input DMAs (issue via both HWDGE engines) -------------
            for k in range(KT):
                nc.sync.dma_start(out=xt[k][:], in_=xT[128 * k:128 * (k + 1), :])
                nc.scalar.dma_start(out=wq_s[k][:], in_=wq[128 * k:128 * (k + 1), :])
                nc.sync.dma_start(out=wk_s[k][:], in_=wk[128 * k:128 * (k + 1), :])
                nc.scalar.dma_start(out=wvg_s[k][:], in_=wvg[128 * k:128 * (k + 1), :])
            for k in range(HPC):
                nc.sync.dma_start(out=wo_s[k][:], in_=wo[64 * k:64 * (k + 1), :])
            for k in range(2):
                nc.sync.dma_start(
                    out=bq_s[:, k:k + 1], in_=bq[128 * k:128 * (k + 1)][:, None])
                nc.sync.dma_start(
                    out=bk_s[:, k:k + 1], in_=bk[128 * k:128 * (k + 1)][:, None])
            nc.sync.dma_start(out=bvg_s[:], in_=bvg[None, :].to_broadcast((128, VW)))
            nc.sync.dma_start(out=em_s[:], in_=emask[:, :])
            nc.sync.dma_start(out=md_s[:], in_=md[:, :])

            # ---- Q^T / K^T projections ---------------------------------
            # qt[m][:, c] = (Wq[:, 128m:128m+128]).T @ xT chunk  + bq
            for wsrc, bsrc, dst in ((wq_s, bq_s, qt), (wk_s, bk_s, kt)):
                for m in range(2):
                    for c in range(NCH):
                        ps = PS.tile([128, CH], F32, name="mm", tag="ss")
                        for k in range(KT):
                            nc.tensor.matmul(
                                ps[:],
                                wsrc[k][:, 128 * m:128 * (m + 1)],
                                xt[k][:, CH * c:CH * (c + 1)],
                                start=(k == 0), stop=(k == KT - 1))
                        nc.vector.tensor_scalar_add(
                            dst[m][:, CH * c:CH * (c + 1)], ps[:],
                            bsrc[:, m:m + 1])

            # ---- V (+gate) projection, bias cols -----------------------
            for t in range(LT):
                ps = PS.tile([128, CH], F32, name="mm", tag="ss")
                pv = ps[:, 0:VW]
                for k in range(KT):
                    nc.tensor.matmul(
                        pv,
                        xt[k][:, 128 * t:128 * (t + 1)],
                        wvg_s[k][:],
                        start=(k == 0), stop=(k == KT - 1))
                # gate: sigmoid(x@Wg + bg), then bias col = em*gate + md
                nc.vector.tensor_add(
                    gp[:], ps[:, GW:VW], bvg_s[:, GW:VW])
                nc.scalar.activation(gs[:], gp[:], AF.Exp, scale=-1.0)
                nc.vector.tensor_scalar_add(gs[:], gs[:], 1.0)
                nc.vector.reciprocal(gs[:], gs[:])
                nc.vector.scalar_tensor_tensor(
                    out=bias_c[:, HPC * t:HPC * (t + 1)],
                    in0=gs[:], scalar=em_s[:, t:t + 1],
                    in1=md_s[:, HPC * t:HPC * (t + 1)],
                    op0=AL.mult, op1=AL.add)
                # V(+bias) -> SBUF, then ones column per head
                nc.vector.tensor_add(
                    v[t][:, 0:GW], pv[:, 0:GW], bvg_s[:, 0:GW])
                # ones columns come via bvg (host sets bvg[65h+64]=1)

            # ---- attention, S^T layout ---------------------------------
            # (PV)^T accumulators in f32 (normalized in f32, cast to bf16)
            ot = [P1.tile([64, L], F32, name=f"ot{h}") for h in range(HPC)]
            otb = [P1.tile([64, L], BF16, name=f"xt{h}") for h in range(HPC)]
            cs2 = P1.tile([65, 2 * L], F32, name="cs2")
            SC = 1.0 / float(np.sqrt(HD))
            for hp in range(2):
                ha, hb = 2 * hp, 2 * hp + 1
                for cp in range(NCH // 2):
                    c0 = 2 * cp
                    pvs = {}
                    for h in (ha, hb):
                        for j in range(2):
                            pvs[(h, j)] = PS.tile(
                                [128, CH], F32, name="pv", tag="pv", bufs=4)
                    for m in range(LT):
                        ss = {}
                        for h in (ha, hb):
                            ss[h] = PS.tile([128, 2 * CH], F32, name="ss2",
                                            tag="ss")
                        # S matmuls interleaved a/b: adjacent mms hit
                        # different PE row groups and run concurrently
                        for j in range(2):
                            for h in (ha, hb):
                                hf = 64 * (h % 2)
                                nc.tensor.matmul(
                                    ss[h][:, CH * j:CH * (j + 1)],
                                    kt[hp][hf:hf + 64, 128 * m:128 * (m + 1)],
                                    qt[hp][hf:hf + 64,
                                           CH * (c0 + j):CH * (c0 + j + 1)],
                                    start=True, stop=True)
                        esx = {}
                        for h in (ha, hb):
                            es2 = ES.tile([128, 2 * CH], BF16, name="es")
                            nc.scalar.activation(
                                es2[:], ss[h][:], AF.Exp,
                                bias=bias_c[:, HPC * m + h:HPC * m + h + 1],
                                scale=SC)
                            esx[h] = es2
                        for j in range(2):
                            for h in (ha, hb):
                                nc.tensor.matmul(
                                    pvs[(h, j)][0:HD + 1, :],
                                    v[m][:, HW_ * h:HW_ * (h + 1)],
                                    esx[h][:, CH * j:CH * (j + 1)],
                                    start=(m == 0), stop=(m == LT - 1))
                    for h in (ha, hb):
                        for j in range(2):
                            pv = pvs[(h, j)]
                            cc = c0 + j
                            nc.vector.tensor_copy(
                                cs2[HD:HD + 1, L * (h % 2) + CH * cc:
                                    L * (h % 2) + CH * (cc + 1)],
                                pv[HD:HD + 1, :])
                            nc.vector.tensor_copy(
                                ot[h][0:HD, CH * cc:CH * (cc + 1)], pv[0:HD, :])
                # normalizer chain for this pair overlaps the next pair's
                # attention: colsums -> DRAM -> transposed -> 1/x -> DRAM
                # -> partition-broadcast -> scale+cast ot -> otb (bf16)
                nc.sync.dma_start(out=csd[ha, :][None, :], in_=cs2[HD:HD + 1, 0:L])
                nc.sync.dma_start(out=csd[hb, :][None, :], in_=cs2[HD:HD + 1, L:2 * L])
                for h in (ha, hb):
                    dsrc = csd[h, :][None, :].rearrange("a (i q) -> a q i", q=128)
                    nc.sync.dma_start(out=cst[:, 16 * h:16 * (h + 1)], in_=dsrc)
                nc.vector.reciprocal(rt[:, 32 * hp:32 * (hp + 1)],
                                     cst[:, 32 * hp:32 * (hp + 1)])
                for h in (ha, hb):
                    ddst = rscr[h, :][None, :].rearrange("a (i q) -> a q i", q=128)
                    nc.sync.dma_start(out=ddst, in_=rt[:, 16 * h:16 * (h + 1)])
                for h in (ha, hb):
                    rb = RB.tile([64, L], F32, name="rb")
                    nc.sync.dma_start(
                        out=rb[:], in_=rscr[h, :][None, :].to_broadcast((64, L)))
                    for c in range(NCH):
                        nc.vector.tensor_mul(
                            otb[h][0:HD, CH * c:CH * (c + 1)],
                            ot[h][0:HD, CH * c:CH * (c + 1)],
                            rb[:, CH * c:CH * (c + 1)])

            # ---- out-projection (partial over this core's 256 dims) ----
            stage_tags = ["qt0", "qt1", "kt0", "kt1"]
            for t in range(LT):
                for n in range(2):
                    ps = PS.tile([128, CH], F32, name="mm", tag="ss")
                    for k in range(HPC):
                        nc.tensor.matmul(
                            ps[:],
                            otb[k][0:HD, 128 * t:128 * (t + 1)],
                            wo_s[k][0:HD, CH * n:CH * (n + 1)],
                            start=(k == 0), stop=(k == HPC - 1))
                    idx = 2 * t + n               # 0..31
                    stage = P1.tile([128, CH], F32, name=stage_tags[idx % 4])
                    nc.vector.tensor_copy(stage[:], ps[:])
                    nc.sync.dma_start(
                        out=out[128 * t:128 * (t + 1), CH * n:CH * (n + 1)],
                        in_=stage[:])

    nc.finalize()
    return nc


def _make_runner():
    """Compile once; return f(in_maps) -> list of per-core output dicts.

    Same execution path as concourse.bass_utils.run_bass_kernel_spmd under
    axon (bass2jax custom-call via PJRT), but with the jitted executable
    cached so repeated calls don't recompile.
    """
    import jax
    from jax.experimental.shard_map import shard_map
    from jax.sharding import Mesh, PartitionSpec
    from concourse import bass2jax, mybir

    nc = _build()
    bass2jax.install_neuronx_cc_hook()

    partition_name = nc.partition_id_tensor.name if nc.partition_id_tensor else None
    in_names, out_names, out_avals, zero_outs = [], [], [], []
    for alloc in nc.m.functions[0].allocations:
        if not isinstance(alloc, mybir.MemoryLocationSet):
            continue
        name = alloc.memorylocations[0].name
        if alloc.kind == "ExternalInput":
            if name != partition_name:
                in_names.append(name)
        elif alloc.kind == "ExternalOutput":
            out_names.append(name)
            shape = tuple(alloc.tensor_shape)
            dtype = mybir.dt.np(alloc.dtype)
            out_avals.append(jax.core.ShapedArray(shape, dtype))
            zero_outs.append(np.zeros(shape, dtype))
    n_params = len(in_names)
    n_outs = len(out_avals)
    feed_names = list(in_names) + list(out_names)
    if partition_name is not None:
        feed_names.append(partition_name)
    donate = tuple(range(n_params, n_params + n_outs))

    def _body(*args):
        operands = list(args)
        if partition_name is not None:
            operands.append(bass2jax.partition_id_tensor())
        outs = bass2jax._bass_exec_p.bind(
            *operands,
            out_avals=tuple(out_avals),
            in_names=tuple(feed_names),
            out_names=tuple(out_names),
            lowering_input_output_aliases=(),
            sim_require_finite=True,
            sim_require_nnan=True,
            nc=nc,
        )
        return tuple(outs)

    devices = jax.devices()[:NCORES]
    mesh = Mesh(np.asarray(devices), ("core",))
    sharded = jax.jit(
        shard_map(
            _body, mesh=mesh,
            in_specs=(PartitionSpec("core"),) * (n_params + n_outs),
            out_specs=(PartitionSpec("core"),) * n_outs,
            check_rep=False,
        ),
        donate_argnums=donate, keep_unused=True,
    )

    def run(in_maps):
        gi = [np.concatenate([np.asarray(m[nm]) for m in in_maps], axis=0)
              for nm in in_names]
        go = [np.concatenate([z] * NCORES, axis=0) for z in zero_outs]
        outs = sharded(*gi, *go)
        res = []
        for i in range(NCORES):
            d = {}
            for j, nm in enumerate(out_names):
                n0 = zero_outs[j].shape[0]
                d[nm] = np.asarray(outs[j][i * n0:(i + 1) * n0])
            res.append(d)
        return res

    from jax.sharding import NamedSharding
    shd = NamedSharding(mesh, PartitionSpec("core"))
    gshapes = [(NCORES * z.shape[0],) + z.shape[1:] for z in zero_outs]
    gdtypes = [z.dtype for z in zero_outs]
    make_zeros = jax.jit(
        lambda: tuple(
            jax.numpy.zeros(s, d) for s, d in zip(gshapes, gdtypes)),
        out_shardings=(shd,) * n_outs)

    def run_timed(in_maps, iters=10):
        """Device-resident repeat timing: returns list of per-iter seconds."""
        import time
        gi = [jax.device_put(
            np.concatenate([np.asarray(m[nm]) for m in in_maps], axis=0), shd)
            for nm in in_names]
        jax.block_until_ready(gi)
        ts = []
        for _ in range(iters):
            go = make_zeros()
            jax.block_until_ready(go)
            t0 = time.perf_counter()
            outs = sharded(*gi, *go)
            jax.block_until_ready(outs)
            ts.append(time.perf_counter() - t0)
        return ts

    run.timed = run_timed
    return run


def _shard_inputs(hidden_states, attention_mask, has_error_codes,
                  Wq, bq, Wk, bk, Wv, bv, Wo, bo, diag_bias, Wg, bg):
    import ml_dtypes
    bf16 = ml_dtypes.bfloat16
    f32 = np.float32
    hs = np.asarray(hidden_states, f32)
    am = np.asarray(attention_mask, f32).reshape(B, L)
    ec = np.asarray(has_error_codes).astype(f32)
    Wq, Wk, Wv, Wo = (np.asarray(w, f32) for w in (Wq, Wk, Wv, Wo))
    Wg = np.asarray(Wg, f32)
    bq, bk, bv, bg = (np.asarray(x, f32) for x in (bq, bk, bv, bg))
    diag = np.asarray(diag_bias, f32).reshape(NH)

    in_maps = []
    for core in range(NCORES):
        b, hb = core // 4, core % 4
        heads = range(4 * hb, 4 * hb + 4)
        cols = slice(DPC * hb, DPC * (hb + 1))
        wvg = np.zeros((H, VW), f32)
        bvg = np.zeros((VW,), f32)
        for j, h in enumerate(heads):
            wvg[:, HW_ * j:HW_ * j + HD] = Wv[:, HD * h:HD * (h + 1)]
            bvg[HW_ * j:HW_ * j + HD] = bv[HD * h:HD * (h + 1)]
            wvg[:, GW + j] = Wg[:, h]
            bvg[GW + j] = bg[h]
            bvg[HW_ * j + HD] = 1.0
        mdv = am[b][:, None] + diag[list(heads)][None, :]          # (L, 4)
        in_maps.append({
            "xT": np.ascontiguousarray(hs[b].T).astype(bf16),
            "wq": np.ascontiguousarray(Wq[:, cols]).astype(bf16),
            "wk": np.ascontiguousarray(Wk[:, cols]).astype(bf16),
            "wvg": wvg.astype(bf16),
            "wo": np.ascontiguousarray(Wo[cols, :]).astype(bf16),
            "bq": np.ascontiguousarray(bq[cols]),
            "bk": np.ascontiguousarray(bk[cols]),
            "bvg": bvg,
            "emask": np.ascontiguousarray(ec[b].reshape(LT, 128).T),
            "md": np.ascontiguousarray(
                mdv.reshape(LT, 128, HPC).transpose(1, 0, 2).reshape(128, LT * HPC)),
        })
    return in_maps


def kernel(**inputs) -> np.ndarray:
    global _RUNNER
    if _RUNNER is None:
        _RUNNER = _make_runner()
    in_maps = _shard_inputs(**inputs)
    results = _RUNNER(in_maps)
    bo = np.asarray(inputs["bo"], np.float32)
    out = np.zeros((B, L, H), np.float32)
    for b in range(B):
        acc = np.zeros((L, H), np.float64)
        for j in range(4):
            acc += results[4 * b + j]["out"].astype(np.float64)
        out[b] = (acc + bo.astype(np.float64)).astype(np.float32)
    return out
